# revision 18
# baseline (speedup 1.0000x reference)
"""EdgeOnlyConv GNN message-passing kernel for Trainium2 (8 NeuronCores).

out[e] = concat(x[src[e]], x[dest[e]], edge_attr[e]) @ W.T + b

Strategy (edge-parallel across 8 cores, x & weights replicated):
  Phase A (per core): combined node table C[k] =
    [Ys[2k] | Yd[2k] | Ys[2k+1] | Yd[2k+1]]  (f16, 25088 pair rows x 1KB)
    where Ys = x @ Wsrc.T + b, Yd = x @ Wdst.T. Stored with 512B descriptors.
  Phase B (per core): edges host-sorted into 4 parity groups (src&1, dst&1)
    of 32768 slots each. Per 2048-edge supertile (one parity group):
    - 4 dma_gather calls (2 per endpoint, 1024 int16 pair indices each,
      256B reads via elem_step=512 at the group's static parity offset),
      rotated over 4 SWDGE queues so descriptor-gen and DMA overlap.
    - z = edge_attr @ We.T on PE (f16, host-transposed edge_attr)
    - DVE: q = gsrc + gdst;  out = psum(z) + q  (fused scalar_tensor_tensor)
    - f16 output, big-descriptor store; host undoes the permutation.
"""

import sys
import numpy as np

if "/opt/trn_rl_repo" not in sys.path:
    sys.path.insert(0, "/opt/trn_rl_repo")

P = 128
CALL_IDX = 1024          # indices per dma_gather call (SWDGE ring limit)

N_CORES = 8
N_NODES = 50000
N_IN_NODE = 128
N_IN_EDGE = 64
N_OUT = 128
N_EDGES = 1000000
E_CORE = N_EDGES // N_CORES          # 125000
GROUP_SLOTS = 32768                  # slots per parity group (32 calls)
N_GROUPS = 4
E_PAD = N_GROUPS * GROUP_SLOTS       # 131072
K_SUP = 16                           # 128-edge tiles per supertile
S_SUP = E_PAD // (K_SUP * P)         # 64 supertiles
NODES_PAD = (N_NODES + 255) // 256 * 256   # 50176
A_TILES = NODES_PAD // P             # 392
PAIR_ROWS = NODES_PAD // 2           # 25088


def build_program():
    import concourse.mybir as mybir
    import concourse.tile as tile
    from concourse import bacc

    f16 = mybir.dt.float16
    f32 = mybir.dt.float32
    i16 = mybir.dt.int16
    Copy = mybir.ActivationFunctionType.Copy
    Alu = mybir.AluOpType

    d_comb = 2 * N_OUT                      # 256
    idx_cols = CALL_IDX // 16               # 64 int16 cols per call
    sup_cols = K_SUP * P                    # 2048
    calls_per_side = sup_cols // CALL_IDX   # 2

    nc = bacc.Bacc("TRN2", target_bir_lowering=False, debug=False,
                   num_devices=N_CORES, num_swdge_queues=4)

    xt_d = nc.dram_tensor("xt", [P, NODES_PAD], f16, kind="ExternalInput").ap()
    wct_d = nc.dram_tensor("wct", [N_IN_NODE, d_comb], f16, kind="ExternalInput").ap()
    wet_d = nc.dram_tensor("wet", [N_IN_EDGE, N_OUT], f16, kind="ExternalInput").ap()
    ones_d = nc.dram_tensor("ones", [1, P], f16, kind="ExternalInput").ap()
    brow_d = nc.dram_tensor("brow", [1, d_comb], f16, kind="ExternalInput").ap()
    brow4_d = nc.dram_tensor("brow4", [1, 4 * P], f16, kind="ExternalInput").ap()
    gs_d = nc.dram_tensor("gs", [P, E_PAD // 16], i16, kind="ExternalInput").ap()
    gd_d = nc.dram_tensor("gd", [P, E_PAD // 16], i16, kind="ExternalInput").ap()
    eat_d = nc.dram_tensor("eat", [N_IN_EDGE, E_PAD], f16, kind="ExternalInput").ap()
    out_d = nc.dram_tensor("out", [P, E_PAD], f16, kind="ExternalOutput").ap()
    c_d = nc.dram_tensor("ctab", [PAIR_ROWS, 2 * d_comb], f16, kind="Internal").ap()
    # node-row view for phase A stores: row n = [Ys[n](128) | Yd[n](128)]
    c_nodes = c_d.rearrange("k (j f) -> (k j) f", j=2)

    GRP = 14  # node tiles per phase-A group

    with tile.TileContext(nc) as tc:
        with tc.tile_pool(name="static", bufs=1) as spool:
            wct_sb = spool.tile([N_IN_NODE, d_comb], f16)
            nc.sync.dma_start(wct_sb[:], wct_d[:, :])
            wet_sb = spool.tile([N_IN_EDGE, N_OUT], f16)
            nc.sync.dma_start(wet_sb[:], wet_d[:, :])
            ones_sb = spool.tile([1, P], f16)
            nc.sync.dma_start(ones_sb[:], ones_d[:, :])
            brow_sb = spool.tile([1, d_comb], f16)
            nc.sync.dma_start(brow_sb[:], brow_d[:, :])
            brow4_sb = spool.tile([1, 4 * P], f16)
            nc.sync.dma_start(brow4_sb[:], brow4_d[:, :])
            gs_sb = spool.tile([P, E_PAD // 16], i16)
            nc.sync.dma_start(gs_sb[:], gs_d[:, :])
            gd_sb = spool.tile([P, E_PAD // 16], i16)
            nc.sync.dma_start(gd_sb[:], gd_d[:, :])

            # ---- Phase A: C pair table from xT (feature-major) ----
            with tc.tile_pool(name="a_sbuf", bufs=2) as apool, \
                 tc.tile_pool(name="a_x", bufs=1) as xpool, \
                 tc.tile_pool(name="a_ps", bufs=8, space="PSUM") as apsum:
                xt_sb = xpool.tile([P, NODES_PAD], f16)
                qn = NODES_PAD // 4
                for ci in range(4):
                    nc.sync.dma_start(xt_sb[:, ci * qn:(ci + 1) * qn],
                                      xt_d[:, ci * qn:(ci + 1) * qn])
                for g0 in range(0, A_TILES, GRP):
                    yc_sb = apool.tile([P, GRP, d_comb], f16, tag="yc_sb")
                    for h in range(0, GRP, 2):
                        yc_ps = apsum.tile([P, 2 * d_comb], f32, tag="yc_ps")
                        for j in range(2):
                            i = g0 + h + j
                            nc.tensor.matmul(
                                yc_ps[:, j * d_comb:(j + 1) * d_comb],
                                lhsT=xt_sb[:, i * P:(i + 1) * P],
                                rhs=wct_sb[:], start=True, stop=True)
                        dst = yc_sb.rearrange("p g f -> p (g f)")[
                            :, h * d_comb:(h + 2) * d_comb]
                        if (h // 2) % 2 == 0:
                            nc.scalar.activation(dst, yc_ps[:, :], Copy)
                        else:
                            nc.vector.tensor_copy(dst, yc_ps[:, :])
                    rows = c_nodes[g0 * P:(g0 + GRP) * P, :].rearrange(
                        "(g p) f -> p g f", p=P)
                    nc.sync.dma_start(rows[:, :, :], yc_sb[:, :, :])

            tc.strict_bb_all_engine_barrier()

            # ---- Phase B ----
            with tc.tile_pool(name="b_sbuf", bufs=8) as bpool, \
                 tc.tile_pool(name="b_ps", bufs=8, space="PSUM") as bpsum:
                for s in range(S_SUP):
                    grp = s // (S_SUP // N_GROUPS)
                    sp, dp = (grp >> 1) & 1, grp & 1
                    src_off = sp * 2 * N_OUT            # 0 or 512 (elems)
                    dst_off = N_OUT + dp * 2 * N_OUT    # 128 or 640... see map
                    gsrc = bpool.tile([P, K_SUP, N_OUT], f16, tag="gsrc")
                    gdst = bpool.tile([P, K_SUP, N_OUT], f16, tag="gdst")
                    for c in range(calls_per_side):
                        c0 = (s * sup_cols // 16) + c * idx_cols
                        q_base = (s * 2 * calls_per_side + 2 * c) % 4
                        nc.gpsimd.dma_gather(
                            out_ap=gsrc[:, c * 8:(c + 1) * 8, :],
                            in_ap=c_d[:, src_off:src_off + N_OUT],
                            idxs_ap=gs_sb[:, c0:c0 + idx_cols],
                            num_idxs=CALL_IDX, num_idxs_reg=CALL_IDX,
                            elem_size=N_OUT, elem_step=2 * d_comb,
                            queue_num=q_base)
                        nc.gpsimd.dma_gather(
                            out_ap=gdst[:, c * 8:(c + 1) * 8, :],
                            in_ap=c_d[:, dst_off:dst_off + N_OUT],
                            idxs_ap=gd_sb[:, c0:c0 + idx_cols],
                            num_idxs=CALL_IDX, num_idxs_reg=CALL_IDX,
                            elem_size=N_OUT, elem_step=2 * d_comb,
                            queue_num=q_base + 1)
                    eat_sb = bpool.tile([N_IN_EDGE, sup_cols], f16, tag="eat_sb")
                    nc.sync.dma_start(
                        eat_sb[:, :],
                        eat_d[:, s * sup_cols:(s + 1) * sup_cols])
                    q = bpool.tile([P, K_SUP, N_OUT], f16, tag="q")
                    nc.vector.tensor_add(q[:, :, :], gsrc[:, :, :], gdst[:, :, :])
                    outsb = bpool.tile([P, K_SUP, N_OUT], f16, tag="outsb")
                    for b in range(K_SUP // 4):
                        z_ps = bpsum.tile([P, 4 * P], f32, tag="z_ps")
                        nc.tensor.matmul(
                            z_ps[:, :], lhsT=ones_sb[:, :], rhs=brow4_sb[:, :],
                            start=True, stop=False)
                        for j in range(4):
                            nc.tensor.matmul(
                                z_ps[:, j * P:(j + 1) * P],
                                lhsT=eat_sb[:, (b * 4 + j) * P:(b * 4 + j + 1) * P],
                                rhs=wet_sb[:], start=False, stop=(j == 3),
                                skip_group_check=True)
                        nc.vector.scalar_tensor_tensor(
                            outsb[:, b * 4:(b + 1) * 4, :],
                            in0=z_ps[:, :], scalar=1.0,
                            in1=q[:, b * 4:(b + 1) * 4, :],
                            op0=Alu.bypass, op1=Alu.add)
                    nc.sync.dma_start(
                        out_d[:, s * sup_cols:(s + 1) * sup_cols],
                        outsb.rearrange("p t o -> p (t o)")[:, :])

    nc.compile()
    return nc


def prep_inputs(x, edge_index, edge_attr, W, b):
    """Host-side prep: parity-group sort per core, f16 casts, layouts."""
    x = np.asarray(x, dtype=np.float32)
    edge_index = np.asarray(edge_index)
    edge_attr = np.asarray(edge_attr, dtype=np.float32)
    W = np.asarray(W, dtype=np.float32)
    b = np.asarray(b, dtype=np.float32)

    d_node = x.shape[1]
    xt = np.zeros((P, NODES_PAD), dtype=np.float16)
    xt[:, :x.shape[0]] = x.T.astype(np.float16)
    wct = np.ascontiguousarray(np.concatenate(
        [W[:, :d_node].T, W[:, d_node:2 * d_node].T], axis=1)).astype(np.float16)
    wet = np.ascontiguousarray(W[:, 2 * d_node:].T).astype(np.float16)
    ones = np.ones((1, P), dtype=np.float16)
    brow = np.zeros((1, 2 * N_OUT), dtype=np.float16)
    brow[0, :N_OUT] = b.astype(np.float16)
    brow4 = np.tile(b.astype(np.float16), 4).reshape(1, 4 * P)

    src = np.ascontiguousarray(edge_index[0]).astype(np.int32)
    dst = np.ascontiguousarray(edge_index[1]).astype(np.int32)

    in_maps = []
    perms = []
    for c in range(N_CORES):
        lo, hi = c * E_CORE, (c + 1) * E_CORE
        sc, dc = src[lo:hi], dst[lo:hi]
        grp = (sc & 1) * 2 + (dc & 1)
        slot_to_edge = np.full(E_PAD, -1, dtype=np.int32)
        gs = np.zeros(E_PAD, dtype=np.int16)
        gd = np.zeros(E_PAD, dtype=np.int16)
        for g in range(N_GROUPS):
            idx_e = np.nonzero(grp == g)[0]
            n = idx_e.size
            assert n <= GROUP_SLOTS, f"parity group overflow: {n}"
            base = g * GROUP_SLOTS
            slot_to_edge[base:base + n] = idx_e
            gs[base:base + n] = (sc[idx_e] >> 1).astype(np.int16)
            gd[base:base + n] = (dc[idx_e] >> 1).astype(np.int16)
        valid = slot_to_edge >= 0
        ea_slot = np.zeros((E_PAD, N_IN_EDGE), dtype=np.float16)
        ea_slot[valid] = edge_attr[lo + slot_to_edge[valid]].astype(np.float16)
        eat = np.ascontiguousarray(ea_slot.T)
        in_maps.append({
            "xt": xt, "wct": wct, "wet": wet, "ones": ones, "brow": brow,
            "brow4": brow4,
            "gs": _idx_wrap16(gs, CALL_IDX), "gd": _idx_wrap16(gd, CALL_IDX),
            "eat": eat,
        })
        perms.append(slot_to_edge)
    return in_maps, perms


def _idx_wrap16(seq_i16, n_idx):
    """Pack a flat int16 index sequence into the dma_gather SBUF layout:
    index i of each n_idx-call at (partition i%16, column i//16), replicated
    to 8x16 partition rows."""
    cols = n_idx // 16
    blocks = seq_i16.reshape(-1, cols, 16)
    arr = blocks.transpose(0, 2, 1).reshape(-1, 16, cols)
    out = np.concatenate([np.tile(a, (8, 1)) for a in arr], axis=1)
    return np.ascontiguousarray(out)


def unpack_outputs(res, perms):
    outs = []
    for c in range(N_CORES):
        o = res.results[c]["out"]                       # [128, E_PAD] f16
        # cols are (s, t, chan); slot = s*2048 + t*128 + p
        rows = np.ascontiguousarray(
            o.reshape(P, S_SUP, K_SUP, N_OUT).transpose(1, 2, 0, 3)
            .reshape(E_PAD, N_OUT))
        slot_to_edge = perms[c]
        valid = slot_to_edge >= 0
        oc = np.empty((E_CORE, N_OUT), dtype=np.float32)
        oc[slot_to_edge[valid]] = rows[valid].astype(np.float32)
        outs.append(oc)
    return np.concatenate(outs, axis=0)


_NC_CACHE = {}


def _get_program():
    key = "full"
    if key not in _NC_CACHE:
        _NC_CACHE[key] = build_program()
    return _NC_CACHE[key]


def run_on_hw(in_maps, nc=None, trace=False):
    from concourse import bass_utils
    if nc is None:
        nc = _get_program()
    kw = {}
    if trace:
        _install_profile_hook(bass_utils)
        kw["trace"] = True
    res = bass_utils.run_bass_kernel_spmd(
        nc, in_maps, core_ids=list(range(N_CORES)), **kw)
    return res


def _install_profile_hook(bass_utils):
    """Inject the NTFF profile hook missing from this image's antenv."""
    import types
    if "antenv.axon_hooks" in sys.modules:
        return
    try:
        from trn_agent_boot.trn_boot import _ntff_profile_via_ctypes
        hook = _ntff_profile_via_ctypes("/opt/axon/libaxon_pjrt.so")
    except Exception:
        hook = None
    mod = types.ModuleType("antenv.axon_hooks")
    mod.get_axon_ntff_profile_hook = lambda: hook
    mod.set_axon_ntff_profile_hook = lambda h: None
    sys.modules["antenv.axon_hooks"] = mod
    bass_utils.upload_artifacts = lambda tmpdir: f"file://{tmpdir}"


def kernel(x, edge_index, edge_attr, W, b):
    in_maps, perms = prep_inputs(x, edge_index, edge_attr, W, b)
    res = run_on_hw(in_maps)
    return unpack_outputs(res, perms)
